# revision 4
# baseline (speedup 1.0000x reference)
"""GCN+ReLU 2-layer kernel for Trainium2, 8 NeuronCores.

Strategy (dst-partitioned graph):
  - Nodes are split into 8 contiguous slices; each core owns the edges whose
    dst lands in its slice (host groups+sorts edges by dst once, in numpy).
  - segment_sum per 128-dst tile via one-hot selection matmuls in bf16:
    gathered src rows X_j (bf16) accumulate X_j^T @ S_j into PSUM, where
    S_j[p,q] = (oh[p,j]==q) is built on-device (iota + is_equal, int16 in,
    bf16 out, one batched DVE op per dst tile). Aggregation runs on the
    *input* features (linearity), so the dense W matmul runs once per tile.
  - dma_gather instructions are batched: one gather per (7-tile group, bank)
    instead of per (tile, bank) — the ~1us fixed SWDGE descriptor-gen cost
    on the Pool engine dominated the old per-tile version.
  - Gather sources are bf16 (h converted on host for layer 0; layer-0 output
    stored + AllGathered as bf16 for layer 1). int16 gather indices limit
    rows to 32k, so sources are split in 4 banks; edges are grouped per
    (group, bank, tile). Both layers share the same indices.
  - Residual branch input for layer 0 comes in host-transposed ([feat, node])
    so no per-tile PE transpose is needed; layer 1 reads the resident xT.
  - Layer outputs live in SBUF transposed [feat, node] so BatchNorm scale/
    shift are per-partition ops; global BN stats via a [128,2] AllReduce.
  - Uniform SPMD program: every core runs the identical instruction stream;
    per-core data (edge indices, one-hot ids) comes in as inputs. Per
    (tile, bank) block counts are padded across cores to a common count
    (pad slots gather row 0; their one-hot id is -1 so the selection
    matrix kills their contribution exactly).
"""
import os
import sys
sys.path.insert(0, '/opt/trn_rl_repo')

from contextlib import ExitStack

import numpy as np
import ml_dtypes

import concourse.bass as bass
import concourse.bacc as bacc_mod
import concourse.mybir as mybir
from concourse import bass_utils
from concourse.tile import TileContext

P = 128
D = 128
N_CORES = 8
N_BANKS = 4
G_TILES = 7          # dst tiles per gather group
BN_EPS = 1e-5

F32 = mybir.dt.float32
BF16 = mybir.dt.bfloat16
I32 = mybir.dt.int32
I16 = mybir.dt.int16
Alu = mybir.AluOpType
Act = mybir.ActivationFunctionType

ABLATE = set()


def _preprocess(src, dst, N, n_cores):
    """Group edges by (dst slice, dst tile, src bank); pad per (tile, bank)
    to a cross-core-uniform block count.

    Column orders:
      idx16 [128, totblk*8]: blocks ordered (group, bank, tile, blk) —
        matches the batched per-(group,bank) dma_gather layout.
      oh [128, totblk] int16: blocks ordered (group, tile, bank, blk) —
        per-tile contiguous for one batched S build per tile.
    """
    NPC = N // n_cores
    T_NODE = -(-NPC // P)
    bank_rows = -(-N // N_BANKS)
    n_groups = -(-T_NODE // G_TILES)

    order = np.argsort(dst, kind="stable")
    src_s = src[order].astype(np.int64)
    dst_s = dst[order].astype(np.int64)

    core_lo = np.searchsorted(dst_s, np.arange(n_cores) * NPC)
    core_hi = np.searchsorted(dst_s, (np.arange(n_cores) + 1) * NPC)

    per = [[None] * T_NODE for _ in range(n_cores)]
    for c in range(n_cores):
        s_c = src_s[core_lo[c]:core_hi[c]]
        dl_c = dst_s[core_lo[c]:core_hi[c]] - c * NPC
        t_lo = np.searchsorted(dl_c, np.arange(T_NODE) * P)
        t_hi = np.searchsorted(dl_c, (np.arange(T_NODE) + 1) * P)
        for t in range(T_NODE):
            s_t = s_c[t_lo[t]:t_hi[t]]
            d_t = dl_c[t_lo[t]:t_hi[t]] - t * P
            b_t = s_t // bank_rows
            o = np.argsort(b_t, kind="stable")
            s_t, d_t, b_t = s_t[o], d_t[o], b_t[o]
            lo = np.searchsorted(b_t, np.arange(N_BANKS))
            hi = np.searchsorted(b_t, np.arange(N_BANKS) + 1)
            per[c][t] = [(s_t[lo[b]:hi[b]] - b * bank_rows,
                          d_t[lo[b]:hi[b]]) for b in range(N_BANKS)]

    nblk = [[0] * N_BANKS for _ in range(T_NODE)]
    for t in range(T_NODE):
        for b in range(N_BANKS):
            m = max(len(per[c][t][b][0]) for c in range(n_cores))
            nblk[t][b] = -(-m // P)

    totblk = sum(sum(r) for r in nblk)
    totcols = totblk * 8

    # idx col start per (g, b): running counter in (g, b, t, blk) order
    # oh col start per (g, t, b): running counter in (g, t, b, blk) order
    idx16_l, oh_l = [], []
    for c in range(n_cores):
        idx16 = np.zeros((P, totcols), np.int16)
        oh = np.full((P, totblk), -1, np.int16)
        icol = 0
        ocol = 0
        for g in range(n_groups):
            tiles = range(g * G_TILES, min((g + 1) * G_TILES, T_NODE))
            for b in range(N_BANKS):
                for t in tiles:
                    nb = nblk[t][b]
                    if nb == 0:
                        continue
                    ni = nb * P
                    s_tb, _ = per[c][t][b]
                    arr = np.zeros(ni, np.int64)
                    arr[:len(s_tb)] = s_tb
                    tile16 = arr.reshape(ni // 16, 16).T.astype(np.int16)
                    idx16[:, icol:icol + nb * 8] = np.tile(tile16, (8, 1))
                    icol += nb * 8
            for t in tiles:
                for b in range(N_BANKS):
                    nb = nblk[t][b]
                    if nb == 0:
                        continue
                    ni = nb * P
                    _, d_tb = per[c][t][b]
                    ohv = np.full(ni, -1, np.int64)
                    ohv[:len(d_tb)] = d_tb
                    oh[:, ocol:ocol + nb] = ohv.reshape(nb, P).T.astype(np.int16)
                    ocol += nb
        assert icol == totcols and ocol == totblk
        idx16_l.append(idx16)
        oh_l.append(oh)

    meta = dict(NPC=NPC, T_NODE=T_NODE, bank_rows=bank_rows,
                totblk=totblk, totcols=totcols, n_groups=n_groups)
    return idx16_l, oh_l, nblk, meta


def _build(N, nblk, n_cores):
    NPC = N // n_cores
    T_NODE = -(-NPC // P)
    NPC_PAD = T_NODE * P
    bank_rows = -(-N // N_BANKS)
    totblk = sum(sum(r) for r in nblk)
    totcols = totblk * 8
    n_groups = -(-T_NODE // G_TILES)
    groups = [list(range(n_cores))]
    n_last = NPC - (T_NODE - 1) * P

    def grp_tiles(g):
        return list(range(g * G_TILES, min((g + 1) * G_TILES, T_NODE)))

    # blocks per (group, bank) and max totals for tile sizing
    gnb = [[sum(nblk[t][b] for t in grp_tiles(g)) for b in range(N_BANKS)]
           for g in range(n_groups)]
    GBLK_MAX = max(sum(gnb[g]) for g in range(n_groups))
    SMAX = max(sum(nblk[t]) for t in range(T_NODE))

    nc = bacc_mod.Bacc(num_devices=n_cores, num_swdge_queues=4)

    hgb = nc.dram_tensor("hgb", [N, D], BF16, kind="ExternalInput")
    hst = nc.dram_tensor("hst", [P, NPC_PAD], F32, kind="ExternalInput")
    i16d = nc.dram_tensor("i16", [P, totcols], I16, kind="ExternalInput")
    ohd = nc.dram_tensor("oh", [P, totblk], I16, kind="ExternalInput")
    wb0d = nc.dram_tensor("wb0", [D, D], BF16, kind="ExternalInput")
    wr0d = nc.dram_tensor("wr0", [D, D], F32, kind="ExternalInput")
    wb1d = nc.dram_tensor("wb1", [D, D], BF16, kind="ExternalInput")
    wr1d = nc.dram_tensor("wr1", [D, D], F32, kind="ExternalInput")
    bsd = nc.dram_tensor("bs", [D, 8], F32, kind="ExternalInput")
    idnd = nc.dram_tensor("idn", [P, P], F32, kind="ExternalInput")
    yd = nc.dram_tensor("y", [NPC, D], F32, kind="ExternalOutput")

    xb = nc.dram_tensor("xb", [NPC, D], BF16)
    xg = nc.dram_tensor("xg", [n_cores * NPC, D], BF16, addr_space="Shared")
    sti = [nc.dram_tensor(f"sti{i}", [P, 2], F32) for i in range(2)]
    sto = [nc.dram_tensor(f"sto{i}", [P, 2], F32, addr_space="Shared")
           for i in range(2)]

    with TileContext(nc) as tc, ExitStack() as ctx:
        const = ctx.enter_context(tc.tile_pool(name="const", bufs=1))
        big = ctx.enter_context(tc.tile_pool(name="big", bufs=1))
        gpool = ctx.enter_context(tc.tile_pool(name="gp", bufs=2))
        hpool = ctx.enter_context(tc.tile_pool(name="hp", bufs=2))
        spool = ctx.enter_context(tc.tile_pool(name="sp", bufs=2))
        small = ctx.enter_context(tc.tile_pool(name="sm", bufs=4))
        pagg = ctx.enter_context(tc.tile_pool(name="pagg", bufs=2, space="PSUM"))
        pmm = ctx.enter_context(tc.tile_pool(name="pmm", bufs=2, space="PSUM"))
        pres = ctx.enter_context(tc.tile_pool(name="pres", bufs=2, space="PSUM"))

        def ct(shape, dtype, srcap=None, name=None):
            t = const.tile(shape, dtype, tag=name)
            if srcap is not None:
                nc.sync.dma_start(out=t[:], in_=srcap)
            return t

        wb0_t = ct([D, D], BF16, wb0d[:, :], "wb0")
        wr0_t = ct([D, D], F32, wr0d[:, :], "wr0")
        wb1_t = ct([D, D], BF16, wb1d[:, :], "wb1")
        wr1_t = ct([D, D], F32, wr1d[:, :], "wr1")
        bias_t = ct([D, 8], F32, bsd[:, :], "bs")
        ident_t = ct([P, P], F32, idnd[:, :], "idn")
        oh_t = ct([P, totblk], I16, ohd[:, :], "oh")
        i16_t = ct([P, totcols], I16, i16d[:, :], "i16")
        iota_t = ct([P, P], I16, None, "iota")
        nc.gpsimd.iota(iota_t[:], pattern=[[1, P]], base=0, channel_multiplier=0)
        eps_t = ct([P, 1], F32, None, "eps")
        nc.vector.memset(eps_t[:], BN_EPS)

        xT = big.tile([P, NPC_PAD], F32, tag="xT")   # resident transposed acts

        scol = [ct([P, T_NODE], F32, None, f"scol{i}") for i in range(2)]
        qcol = [ct([P, T_NODE], F32, None, f"qcol{i}") for i in range(2)]

        qctr = [0]

        def layer(li, gsrc, wb_t, wr_t, bcol, brcol, gcol, becol, out_dram):
            icol = 0   # idx col cursor, (g, b, t, blk) order
            ocol = 0   # oh col cursor, (g, t, b, blk) order
            for g in range(n_groups):
                tiles = grp_tiles(g)
                gt = gpool.tile([P, GBLK_MAX, D], BF16, tag="g")
                if li == 0:
                    hsg = hpool.tile([P, G_TILES * P], F32, tag="hs")
                    nc.sync.dma_start(
                        out=hsg[:, :len(tiles) * P],
                        in_=hst[:, tiles[0] * P:tiles[0] * P + len(tiles) * P])
                # one gather per bank for the whole group
                boff = 0
                bank_pos = [0] * N_BANKS   # block start of bank b in gt
                split = os.environ.get("K_SPLIT_GATHER")
                for b in range(N_BANKS):
                    bank_pos[b] = boff
                    lo = b * bank_rows
                    hi = min(N, lo + bank_rows)
                    if split:
                        chunks = [nblk[t][b] for t in tiles]
                    else:
                        chunks = [gnb[g][b]]
                    for nb in chunks:
                        if nb == 0:
                            continue
                        ni = nb * P
                        nc.gpsimd.dma_gather(
                            out_ap=gt[:, boff:boff + nb, :],
                            in_ap=gsrc[lo:hi, :],
                            idxs_ap=i16_t[:, icol:icol + nb * 8],
                            num_idxs=ni,
                            num_idxs_reg=ni,
                            elem_size=D,
                            queue_num=qctr[0] % 4,
                        )
                        qctr[0] += 1
                        icol += nb * 8
                        boff += nb
                # gt block position of (t, b, j): bank_pos[b] + (tiles
                # before t in this group's bank b) + j
                bcur = list(bank_pos)
                tile_blocks = {}
                for b in range(N_BANKS):
                    for t in tiles:
                        for j in range(nblk[t][b]):
                            tile_blocks.setdefault(t, []).append(bcur[b])
                            bcur[b] += 1
                for t in tiles:
                    tnb = sum(nblk[t])
                    S2 = spool.tile([P, SMAX, P], BF16, tag="S")
                    nc.vector.tensor_tensor(
                        out=S2[:, :tnb, :],
                        in0=oh_t[:, ocol:ocol + tnb].unsqueeze(2)
                            .to_broadcast([P, tnb, P]),
                        in1=iota_t[:, :].unsqueeze(1)
                            .to_broadcast([P, tnb, P]),
                        op=Alu.is_equal,
                    )
                    pa = pagg.tile([P, P], F32, tag="pa")
                    for k, pos in enumerate(tile_blocks[t]):
                        nc.tensor.matmul(pa[:], lhsT=gt[:, pos, :],
                                         rhs=S2[:, k, :],
                                         start=(k == 0), stop=(k == tnb - 1))
                    ocol += tnb
                    aggT = small.tile([P, P], BF16, tag="aggT")
                    nc.scalar.activation(aggT[:], pa[:], Act.Copy)
                    pm = pmm.tile([P, P], F32, tag="pm")
                    nc.tensor.matmul(pm[:], lhsT=wb_t[:], rhs=aggT[:],
                                     start=True, stop=True)
                    if li == 0:
                        hT = hsg[:, (t - tiles[0]) * P:(t - tiles[0] + 1) * P]
                    else:
                        hT = xT[:, t * P:(t + 1) * P]
                    newt = small.tile([P, P], F32, tag="newt")
                    nc.scalar.activation(newt[:], pm[:], Act.Relu,
                                         bias=bias_t[:, bcol:bcol + 1])
                    pr = pres.tile([P, P], F32, tag="pq")
                    nc.tensor.matmul(pr[:], lhsT=wr_t[:], rhs=hT,
                                     start=True, stop=True)
                    rest = small.tile([P, P], F32, tag="rest")
                    nc.scalar.activation(rest[:], pr[:], Act.Relu,
                                         bias=bias_t[:, brcol:brcol + 1])
                    ov = xT[:, t * P:(t + 1) * P]
                    if t == T_NODE - 1 and n_last < P:
                        nc.vector.scalar_tensor_tensor(
                            out=ov, in0=newt[:], scalar=0.0, in1=rest[:],
                            op0=Alu.add, op1=Alu.add)
                        nc.vector.memset(xT[:, t * P + n_last:(t + 1) * P], 0.0)
                        nc.vector.reduce_sum(out=scol[li][:, t:t + 1], in_=ov,
                                             axis=mybir.AxisListType.X)
                    else:
                        nc.vector.scalar_tensor_tensor(
                            out=ov, in0=newt[:], scalar=0.0, in1=rest[:],
                            op0=Alu.add, op1=Alu.add,
                            accum_out=scol[li][:, t:t + 1])
                    sq = small.tile([P, P], F32, tag="sq")
                    nc.scalar.activation(sq[:], ov, Act.Square,
                                         accum_out=qcol[li][:, t:t + 1])
            assert ocol == totblk and icol == totcols

            def store_phase(dt):
                for t in range(T_NODE):
                    pt = pres.tile([P, P], F32, tag="pq")
                    nc.tensor.transpose(pt[:], xT[:, t * P:(t + 1) * P],
                                        ident_t[:])
                    stg2 = small.tile([P, P], dt, tag=f"stage{li}")
                    nc.scalar.activation(stg2[:], pt[:], Act.Copy)
                    nrow = P if t < T_NODE - 1 else n_last
                    nc.sync.dma_start(out=out_dram[t * P:t * P + nrow, :],
                                      in_=stg2[:nrow, :])

            # --- global BN stats ---
            st_sb = small.tile([P, 2], F32, tag="stats")
            nc.vector.reduce_sum(out=st_sb[:, 0:1], in_=scol[li][:],
                                 axis=mybir.AxisListType.X)
            nc.vector.reduce_sum(out=st_sb[:, 1:2], in_=qcol[li][:],
                                 axis=mybir.AxisListType.X)
            nc.sync.dma_start(out=sti[li][:, :], in_=st_sb[:])
            nc.gpsimd.collective_compute(
                "AllReduce", Alu.add, replica_groups=groups,
                ins=[sti[li].ap().opt()], outs=[sto[li].ap().opt()])
            stg = small.tile([P, 2], F32, tag="stg")
            nc.sync.dma_start(out=stg[:], in_=sto[li][:, :])
            mean = small.tile([P, 1], F32, tag="mean")
            nc.vector.tensor_scalar_mul(mean[:], stg[:, 0:1], 1.0 / N)
            ex2 = small.tile([P, 1], F32, tag="ex2")
            nc.vector.tensor_scalar_mul(ex2[:], stg[:, 1:2], 1.0 / N)
            var = small.tile([P, 1], F32, tag="var")
            nc.vector.tensor_tensor(out=var[:], in0=mean[:], in1=mean[:],
                                    op=Alu.mult)
            nc.vector.tensor_tensor(out=var[:], in0=ex2[:], in1=var[:],
                                    op=Alu.subtract)
            sd = small.tile([P, 1], F32, tag="sd")
            nc.scalar.activation(sd[:], var[:], Act.Sqrt, bias=eps_t[:, 0:1])
            rstd = small.tile([P, 1], F32, tag="rstd")
            nc.vector.reciprocal(rstd[:], sd[:])
            scale_t = small.tile([P, 1], F32, tag="scale")
            nc.vector.tensor_tensor(out=scale_t[:],
                                    in0=bias_t[:, gcol:gcol + 1],
                                    in1=rstd[:], op=Alu.mult)
            shift_t = small.tile([P, 1], F32, tag="shift")
            nc.vector.tensor_tensor(out=shift_t[:], in0=mean[:],
                                    in1=scale_t[:], op=Alu.mult)
            nc.vector.tensor_tensor(out=shift_t[:],
                                    in0=bias_t[:, becol:becol + 1],
                                    in1=shift_t[:], op=Alu.subtract)
            # BN apply in place on the resident transposed buffer
            nc.vector.tensor_scalar(
                out=xT[:, :], in0=xT[:, :],
                scalar1=scale_t[:, 0:1], scalar2=shift_t[:, 0:1],
                op0=Alu.mult, op1=Alu.add)
            store_phase(BF16 if li == 0 else F32)

        layer(0, hgb, wb0_t, wr0_t, 0, 1, 2, 3, xb)
        nc.gpsimd.collective_compute(
            "AllGather", Alu.bypass, replica_groups=groups,
            ins=[xb.ap().opt()], outs=[xg.ap().opt()])
        layer(1, xg, wb1_t, wr1_t, 4, 5, 6, 7, yd)
    nc.compile()
    return nc


def _run(inputs, n_cores=N_CORES, trace=False, runner=None):
    h = np.asarray(inputs["h"], np.float32)
    src = np.asarray(inputs["src"])
    dst = np.asarray(inputs["dst"])
    N = h.shape[0]
    NPC = N // n_cores
    idx16_l, oh_l, nblk, meta = _preprocess(src, dst, N, n_cores)
    T_NODE = meta["T_NODE"]
    NPC_PAD = T_NODE * P

    nc = _build(N, nblk, n_cores)

    bs = np.stack([
        np.asarray(inputs["b0"], np.float32),
        np.asarray(inputs["br0"], np.float32),
        np.asarray(inputs["g0"], np.float32),
        np.asarray(inputs["be0"], np.float32),
        np.asarray(inputs["b1"], np.float32),
        np.asarray(inputs["br1"], np.float32),
        np.asarray(inputs["g1"], np.float32),
        np.asarray(inputs["be1"], np.float32),
    ], axis=1)
    idn = np.eye(P, dtype=np.float32)
    hgb = h.astype(ml_dtypes.bfloat16)

    in_maps = []
    for c in range(n_cores):
        hst_c = np.zeros((P, NPC_PAD), np.float32)
        hst_c[:, :NPC] = h[c * NPC:(c + 1) * NPC].T
        in_maps.append({
            "hgb": hgb,
            "hst": hst_c,
            "i16": idx16_l[c],
            "oh": oh_l[c],
            "wb0": np.asarray(inputs["W0"], np.float32).astype(ml_dtypes.bfloat16),
            "wr0": np.asarray(inputs["Wr0"], np.float32),
            "wb1": np.asarray(inputs["W1"], np.float32).astype(ml_dtypes.bfloat16),
            "wr1": np.asarray(inputs["Wr1"], np.float32),
            "bs": bs,
            "idn": idn,
        })

    if runner is not None:
        results, extra = runner(nc, in_maps)
    else:
        res = bass_utils.run_bass_kernel_spmd(
            nc, in_maps, core_ids=list(range(n_cores)), trace=trace)
        results, extra = res.results, res

    xs = [results[c]["y"][:NPC] for c in range(n_cores)]
    out = np.concatenate(xs, axis=0)
    bsz = int(inputs["batch_size"])
    return out.reshape(bsz, -1, D).astype(np.float32), extra


def kernel(**inputs):
    out, _ = _run(inputs, trace=False)
    return out


# revision 5
# speedup vs baseline: 1.0726x; 1.0726x over previous
"""GCN+ReLU 2-layer kernel for Trainium2, 8 NeuronCores.

Strategy (dst-partitioned graph):
  - Nodes are split into 8 contiguous slices; each core owns the edges whose
    dst lands in its slice (host groups+sorts edges by dst once, in numpy).
  - segment_sum per 128-dst tile via one-hot selection matmuls in bf16:
    gathered src rows X_j (bf16) accumulate X_j^T @ S_j into PSUM, where
    S_j[p,q] = (oh[p,j]==q) is built on-device (iota + is_equal, int16 in,
    bf16 out, one batched DVE op per dst tile). Aggregation runs on the
    *input* features (linearity), so the dense W matmul runs once per tile.
  - dma_gather instructions are batched: one gather per (7-tile group, bank)
    instead of per (tile, bank) — the ~1us fixed SWDGE descriptor-gen cost
    on the Pool engine dominated the old per-tile version.
  - Gather sources are bf16 (h converted on host for layer 0; layer-0 output
    stored + AllGathered as bf16 for layer 1). int16 gather indices limit
    rows to 32k, so sources are split in 4 banks; edges are grouped per
    (group, bank, tile). Both layers share the same indices.
  - Residual branch input for layer 0 comes in host-transposed ([feat, node])
    so no per-tile PE transpose is needed; layer 1 reads the resident xT.
  - Layer outputs live in SBUF transposed [feat, node] so BatchNorm scale/
    shift are per-partition ops; global BN stats via a [128,2] AllReduce.
  - Uniform SPMD program: every core runs the identical instruction stream;
    per-core data (edge indices, one-hot ids) comes in as inputs. Per
    (tile, bank) block counts are padded across cores to a common count
    (pad slots gather row 0; their one-hot id is -1 so the selection
    matrix kills their contribution exactly).
"""
import os
import sys
sys.path.insert(0, '/opt/trn_rl_repo')

from contextlib import ExitStack

import numpy as np
import ml_dtypes

import concourse.bass as bass
import concourse.bacc as bacc_mod
import concourse.mybir as mybir
from concourse import bass_utils
from concourse.tile import TileContext

P = 128
D = 128
N_CORES = 8
N_BANKS = 4
G_TILES = 7          # dst tiles per gather group
BN_EPS = 1e-5

F32 = mybir.dt.float32
BF16 = mybir.dt.bfloat16
I32 = mybir.dt.int32
I16 = mybir.dt.int16
Alu = mybir.AluOpType
Act = mybir.ActivationFunctionType

ABLATE = set()


def _preprocess(src, dst, N, n_cores):
    """Group edges by (dst slice, dst tile, src bank); pad per (tile, bank)
    to a cross-core-uniform block count.

    Column orders:
      idx16 [128, totblk*8]: blocks ordered (group, bank, tile, blk) —
        matches the batched per-(group,bank) dma_gather layout.
      oh [128, totblk] int16: blocks ordered (group, tile, bank, blk) —
        per-tile contiguous for one batched S build per tile.
    """
    NPC = N // n_cores
    T_NODE = -(-NPC // P)
    bank_rows = -(-N // N_BANKS)
    n_groups = -(-T_NODE // G_TILES)

    order = np.argsort(dst, kind="stable")
    src_s = src[order].astype(np.int64)
    dst_s = dst[order].astype(np.int64)

    core_lo = np.searchsorted(dst_s, np.arange(n_cores) * NPC)
    core_hi = np.searchsorted(dst_s, (np.arange(n_cores) + 1) * NPC)

    per = [[None] * T_NODE for _ in range(n_cores)]
    for c in range(n_cores):
        s_c = src_s[core_lo[c]:core_hi[c]]
        dl_c = dst_s[core_lo[c]:core_hi[c]] - c * NPC
        t_lo = np.searchsorted(dl_c, np.arange(T_NODE) * P)
        t_hi = np.searchsorted(dl_c, (np.arange(T_NODE) + 1) * P)
        for t in range(T_NODE):
            s_t = s_c[t_lo[t]:t_hi[t]]
            d_t = dl_c[t_lo[t]:t_hi[t]] - t * P
            b_t = s_t // bank_rows
            o = np.argsort(b_t, kind="stable")
            s_t, d_t, b_t = s_t[o], d_t[o], b_t[o]
            lo = np.searchsorted(b_t, np.arange(N_BANKS))
            hi = np.searchsorted(b_t, np.arange(N_BANKS) + 1)
            per[c][t] = [(s_t[lo[b]:hi[b]] - b * bank_rows,
                          d_t[lo[b]:hi[b]]) for b in range(N_BANKS)]

    nblk = [[0] * N_BANKS for _ in range(T_NODE)]
    for t in range(T_NODE):
        for b in range(N_BANKS):
            m = max(len(per[c][t][b][0]) for c in range(n_cores))
            nblk[t][b] = -(-m // P)

    totblk = sum(sum(r) for r in nblk)
    totcols = totblk * 8

    # idx col start per (g, b): running counter in (g, b, t, blk) order
    # oh col start per (g, t, b): running counter in (g, t, b, blk) order
    idx16_l, oh_l = [], []
    for c in range(n_cores):
        idx16 = np.zeros((P, totcols), np.int16)
        oh = np.full((P, totblk), -1, np.int16)
        icol = 0
        ocol = 0
        for g in range(n_groups):
            tiles = range(g * G_TILES, min((g + 1) * G_TILES, T_NODE))
            for b in range(N_BANKS):
                for t in tiles:
                    nb = nblk[t][b]
                    if nb == 0:
                        continue
                    ni = nb * P
                    s_tb, _ = per[c][t][b]
                    arr = np.zeros(ni, np.int64)
                    arr[:len(s_tb)] = s_tb
                    tile16 = arr.reshape(ni // 16, 16).T.astype(np.int16)
                    idx16[:, icol:icol + nb * 8] = np.tile(tile16, (8, 1))
                    icol += nb * 8
            for t in tiles:
                for b in range(N_BANKS):
                    nb = nblk[t][b]
                    if nb == 0:
                        continue
                    ni = nb * P
                    _, d_tb = per[c][t][b]
                    ohv = np.full(ni, -1, np.int64)
                    ohv[:len(d_tb)] = d_tb
                    oh[:, ocol:ocol + nb] = ohv.reshape(nb, P).T.astype(np.int16)
                    ocol += nb
        assert icol == totcols and ocol == totblk
        idx16_l.append(idx16)
        oh_l.append(oh)

    meta = dict(NPC=NPC, T_NODE=T_NODE, bank_rows=bank_rows,
                totblk=totblk, totcols=totcols, n_groups=n_groups)
    return idx16_l, oh_l, nblk, meta


def _build(N, nblk, n_cores):
    NPC = N // n_cores
    T_NODE = -(-NPC // P)
    NPC_PAD = T_NODE * P
    bank_rows = -(-N // N_BANKS)
    totblk = sum(sum(r) for r in nblk)
    totcols = totblk * 8
    n_groups = -(-T_NODE // G_TILES)
    groups = [list(range(n_cores))]
    n_last = NPC - (T_NODE - 1) * P

    def grp_tiles(g):
        return list(range(g * G_TILES, min((g + 1) * G_TILES, T_NODE)))

    # blocks per (group, bank) and max totals for tile sizing
    gnb = [[sum(nblk[t][b] for t in grp_tiles(g)) for b in range(N_BANKS)]
           for g in range(n_groups)]
    GBLK_MAX = max(sum(gnb[g]) for g in range(n_groups))
    SMAX = max(sum(nblk[t]) for t in range(T_NODE))

    nc = bacc_mod.Bacc(num_devices=n_cores, num_swdge_queues=4)

    hgb = nc.dram_tensor("hgb", [N, D], BF16, kind="ExternalInput")
    hst = nc.dram_tensor("hst", [P, NPC_PAD], F32, kind="ExternalInput")
    i16d = nc.dram_tensor("i16", [P, totcols], I16, kind="ExternalInput")
    ohd = nc.dram_tensor("oh", [P, totblk], I16, kind="ExternalInput")
    wb0d = nc.dram_tensor("wb0", [D, D], BF16, kind="ExternalInput")
    wr0d = nc.dram_tensor("wr0", [D, D], F32, kind="ExternalInput")
    wb1d = nc.dram_tensor("wb1", [D, D], BF16, kind="ExternalInput")
    wr1d = nc.dram_tensor("wr1", [D, D], F32, kind="ExternalInput")
    bsd = nc.dram_tensor("bs", [D, 8], F32, kind="ExternalInput")
    idnd = nc.dram_tensor("idn", [P, P], F32, kind="ExternalInput")
    yd = nc.dram_tensor("y", [NPC, D], F32, kind="ExternalOutput")

    xb = nc.dram_tensor("xb", [NPC, D], BF16)
    xg = nc.dram_tensor("xg", [n_cores * NPC, D], BF16, addr_space="Shared")
    sti = [nc.dram_tensor(f"sti{i}", [P, 2], F32) for i in range(2)]
    sto = [nc.dram_tensor(f"sto{i}", [P, 2], F32, addr_space="Shared")
           for i in range(2)]

    with TileContext(nc) as tc, ExitStack() as ctx:
        const = ctx.enter_context(tc.tile_pool(name="const", bufs=1))
        big = ctx.enter_context(tc.tile_pool(name="big", bufs=1))
        gpool = ctx.enter_context(tc.tile_pool(name="gp", bufs=2))
        hpool = ctx.enter_context(tc.tile_pool(name="hp", bufs=2))
        spool = ctx.enter_context(tc.tile_pool(name="sp", bufs=2))
        small = ctx.enter_context(tc.tile_pool(name="sm", bufs=4))
        pagg = ctx.enter_context(tc.tile_pool(name="pagg", bufs=2, space="PSUM"))
        pmm = ctx.enter_context(tc.tile_pool(name="pmm", bufs=2, space="PSUM"))
        pres = ctx.enter_context(tc.tile_pool(name="pres", bufs=2, space="PSUM"))

        def ct(shape, dtype, srcap=None, name=None):
            t = const.tile(shape, dtype, tag=name)
            if srcap is not None:
                nc.sync.dma_start(out=t[:], in_=srcap)
            return t

        wb0_t = ct([D, D], BF16, wb0d[:, :], "wb0")
        wr0_t = ct([D, D], F32, wr0d[:, :], "wr0")
        wb1_t = ct([D, D], BF16, wb1d[:, :], "wb1")
        wr1_t = ct([D, D], F32, wr1d[:, :], "wr1")
        bias_t = ct([D, 8], F32, bsd[:, :], "bs")
        ident_t = ct([P, P], F32, idnd[:, :], "idn")
        oh_t = ct([P, totblk], I16, ohd[:, :], "oh")
        i16_t = ct([P, totcols], I16, i16d[:, :], "i16")
        iota_t = ct([P, P], I16, None, "iota")
        nc.gpsimd.iota(iota_t[:], pattern=[[1, P]], base=0, channel_multiplier=0)
        eps_t = ct([P, 1], F32, None, "eps")
        nc.vector.memset(eps_t[:], BN_EPS)

        xT = big.tile([P, NPC_PAD], F32, tag="xT")   # resident transposed acts

        scol = [ct([P, T_NODE], F32, None, f"scol{i}") for i in range(2)]
        qcol = [ct([P, T_NODE], F32, None, f"qcol{i}") for i in range(2)]

        qctr = [0]

        def layer(li, gsrc, wb_t, wr_t, bcol, brcol, gcol, becol, out_dram):
            icol = 0   # idx col cursor, (g, b, t, blk) order
            ocol = 0   # oh col cursor, (g, t, b, blk) order
            for g in range(n_groups):
                tiles = grp_tiles(g)
                gt = gpool.tile([P, GBLK_MAX, D], BF16, tag="g")
                if li == 0:
                    hsg = hpool.tile([P, G_TILES * P], F32, tag="hs")
                    nc.sync.dma_start(
                        out=hsg[:, :len(tiles) * P],
                        in_=hst[:, tiles[0] * P:tiles[0] * P + len(tiles) * P])
                # one gather per bank for the whole group
                boff = 0
                bank_pos = [0] * N_BANKS   # block start of bank b in gt
                cap = int(os.environ.get("K_GCAP", "16"))
                for b in range(N_BANKS):
                    bank_pos[b] = boff
                    lo = b * bank_rows
                    hi = min(N, lo + bank_rows)
                    left = gnb[g][b]
                    while left > 0:
                        nb = min(left, cap)
                        left -= nb
                        ni = nb * P
                        nc.gpsimd.dma_gather(
                            out_ap=gt[:, boff:boff + nb, :],
                            in_ap=gsrc[lo:hi, :],
                            idxs_ap=i16_t[:, icol:icol + nb * 8],
                            num_idxs=ni,
                            num_idxs_reg=ni,
                            elem_size=D,
                            queue_num=qctr[0] % 4,
                        )
                        qctr[0] += 1
                        icol += nb * 8
                        boff += nb
                # gt block position of (t, b, j): bank_pos[b] + (tiles
                # before t in this group's bank b) + j
                bcur = list(bank_pos)
                tile_blocks = {}
                for b in range(N_BANKS):
                    for t in tiles:
                        for j in range(nblk[t][b]):
                            tile_blocks.setdefault(t, []).append(bcur[b])
                            bcur[b] += 1
                for t in tiles:
                    tnb = sum(nblk[t])
                    S2 = spool.tile([P, SMAX, P], BF16, tag="S")
                    nc.vector.tensor_tensor(
                        out=S2[:, :tnb, :],
                        in0=oh_t[:, ocol:ocol + tnb].unsqueeze(2)
                            .to_broadcast([P, tnb, P]),
                        in1=iota_t[:, :].unsqueeze(1)
                            .to_broadcast([P, tnb, P]),
                        op=Alu.is_equal,
                    )
                    pa = pagg.tile([P, P], F32, tag="pa")
                    for k, pos in enumerate(tile_blocks[t]):
                        nc.tensor.matmul(pa[:], lhsT=gt[:, pos, :],
                                         rhs=S2[:, k, :],
                                         start=(k == 0), stop=(k == tnb - 1))
                    ocol += tnb
                    aggT = small.tile([P, P], BF16, tag="aggT")
                    nc.scalar.activation(aggT[:], pa[:], Act.Copy)
                    pm = pmm.tile([P, P], F32, tag="pm")
                    nc.tensor.matmul(pm[:], lhsT=wb_t[:], rhs=aggT[:],
                                     start=True, stop=True)
                    if li == 0:
                        hT = hsg[:, (t - tiles[0]) * P:(t - tiles[0] + 1) * P]
                    else:
                        hT = xT[:, t * P:(t + 1) * P]
                    newt = small.tile([P, P], F32, tag="newt")
                    nc.scalar.activation(newt[:], pm[:], Act.Relu,
                                         bias=bias_t[:, bcol:bcol + 1])
                    pr = pres.tile([P, P], F32, tag="pq")
                    nc.tensor.matmul(pr[:], lhsT=wr_t[:], rhs=hT,
                                     start=True, stop=True)
                    rest = small.tile([P, P], F32, tag="rest")
                    nc.scalar.activation(rest[:], pr[:], Act.Relu,
                                         bias=bias_t[:, brcol:brcol + 1])
                    ov = xT[:, t * P:(t + 1) * P]
                    if t == T_NODE - 1 and n_last < P:
                        nc.vector.scalar_tensor_tensor(
                            out=ov, in0=newt[:], scalar=0.0, in1=rest[:],
                            op0=Alu.add, op1=Alu.add)
                        nc.vector.memset(xT[:, t * P + n_last:(t + 1) * P], 0.0)
                        nc.vector.reduce_sum(out=scol[li][:, t:t + 1], in_=ov,
                                             axis=mybir.AxisListType.X)
                    else:
                        nc.vector.scalar_tensor_tensor(
                            out=ov, in0=newt[:], scalar=0.0, in1=rest[:],
                            op0=Alu.add, op1=Alu.add,
                            accum_out=scol[li][:, t:t + 1])
                    sq = small.tile([P, P], F32, tag="sq")
                    nc.scalar.activation(sq[:], ov, Act.Square,
                                         accum_out=qcol[li][:, t:t + 1])
            assert ocol == totblk and icol == totcols

            def store_phase(dt):
                for t in range(T_NODE):
                    pt = pres.tile([P, P], F32, tag="pq")
                    nc.tensor.transpose(pt[:], xT[:, t * P:(t + 1) * P],
                                        ident_t[:])
                    stg2 = small.tile([P, P], dt, tag=f"stage{li}")
                    nc.scalar.activation(stg2[:], pt[:], Act.Copy)
                    nrow = P if t < T_NODE - 1 else n_last
                    nc.sync.dma_start(out=out_dram[t * P:t * P + nrow, :],
                                      in_=stg2[:nrow, :])

            # --- global BN stats ---
            st_sb = small.tile([P, 2], F32, tag="stats")
            nc.vector.reduce_sum(out=st_sb[:, 0:1], in_=scol[li][:],
                                 axis=mybir.AxisListType.X)
            nc.vector.reduce_sum(out=st_sb[:, 1:2], in_=qcol[li][:],
                                 axis=mybir.AxisListType.X)
            nc.sync.dma_start(out=sti[li][:, :], in_=st_sb[:])
            nc.gpsimd.collective_compute(
                "AllReduce", Alu.add, replica_groups=groups,
                ins=[sti[li].ap().opt()], outs=[sto[li].ap().opt()])
            stg = small.tile([P, 2], F32, tag="stg")
            nc.sync.dma_start(out=stg[:], in_=sto[li][:, :])
            mean = small.tile([P, 1], F32, tag="mean")
            nc.vector.tensor_scalar_mul(mean[:], stg[:, 0:1], 1.0 / N)
            ex2 = small.tile([P, 1], F32, tag="ex2")
            nc.vector.tensor_scalar_mul(ex2[:], stg[:, 1:2], 1.0 / N)
            var = small.tile([P, 1], F32, tag="var")
            nc.vector.tensor_tensor(out=var[:], in0=mean[:], in1=mean[:],
                                    op=Alu.mult)
            nc.vector.tensor_tensor(out=var[:], in0=ex2[:], in1=var[:],
                                    op=Alu.subtract)
            sd = small.tile([P, 1], F32, tag="sd")
            nc.scalar.activation(sd[:], var[:], Act.Sqrt, bias=eps_t[:, 0:1])
            rstd = small.tile([P, 1], F32, tag="rstd")
            nc.vector.reciprocal(rstd[:], sd[:])
            scale_t = small.tile([P, 1], F32, tag="scale")
            nc.vector.tensor_tensor(out=scale_t[:],
                                    in0=bias_t[:, gcol:gcol + 1],
                                    in1=rstd[:], op=Alu.mult)
            shift_t = small.tile([P, 1], F32, tag="shift")
            nc.vector.tensor_tensor(out=shift_t[:], in0=mean[:],
                                    in1=scale_t[:], op=Alu.mult)
            nc.vector.tensor_tensor(out=shift_t[:],
                                    in0=bias_t[:, becol:becol + 1],
                                    in1=shift_t[:], op=Alu.subtract)
            # BN apply in place on the resident transposed buffer
            nc.vector.tensor_scalar(
                out=xT[:, :], in0=xT[:, :],
                scalar1=scale_t[:, 0:1], scalar2=shift_t[:, 0:1],
                op0=Alu.mult, op1=Alu.add)
            store_phase(BF16 if li == 0 else F32)

        layer(0, hgb, wb0_t, wr0_t, 0, 1, 2, 3, xb)
        nc.gpsimd.collective_compute(
            "AllGather", Alu.bypass, replica_groups=groups,
            ins=[xb.ap().opt()], outs=[xg.ap().opt()])
        layer(1, xg, wb1_t, wr1_t, 4, 5, 6, 7, yd)
    nc.compile()
    return nc


def _run(inputs, n_cores=N_CORES, trace=False, runner=None):
    h = np.asarray(inputs["h"], np.float32)
    src = np.asarray(inputs["src"])
    dst = np.asarray(inputs["dst"])
    N = h.shape[0]
    NPC = N // n_cores
    idx16_l, oh_l, nblk, meta = _preprocess(src, dst, N, n_cores)
    T_NODE = meta["T_NODE"]
    NPC_PAD = T_NODE * P

    nc = _build(N, nblk, n_cores)

    bs = np.stack([
        np.asarray(inputs["b0"], np.float32),
        np.asarray(inputs["br0"], np.float32),
        np.asarray(inputs["g0"], np.float32),
        np.asarray(inputs["be0"], np.float32),
        np.asarray(inputs["b1"], np.float32),
        np.asarray(inputs["br1"], np.float32),
        np.asarray(inputs["g1"], np.float32),
        np.asarray(inputs["be1"], np.float32),
    ], axis=1)
    idn = np.eye(P, dtype=np.float32)
    hgb = h.astype(ml_dtypes.bfloat16)

    in_maps = []
    for c in range(n_cores):
        hst_c = np.zeros((P, NPC_PAD), np.float32)
        hst_c[:, :NPC] = h[c * NPC:(c + 1) * NPC].T
        in_maps.append({
            "hgb": hgb,
            "hst": hst_c,
            "i16": idx16_l[c],
            "oh": oh_l[c],
            "wb0": np.asarray(inputs["W0"], np.float32).astype(ml_dtypes.bfloat16),
            "wr0": np.asarray(inputs["Wr0"], np.float32),
            "wb1": np.asarray(inputs["W1"], np.float32).astype(ml_dtypes.bfloat16),
            "wr1": np.asarray(inputs["Wr1"], np.float32),
            "bs": bs,
            "idn": idn,
        })

    if runner is not None:
        results, extra = runner(nc, in_maps)
    else:
        res = bass_utils.run_bass_kernel_spmd(
            nc, in_maps, core_ids=list(range(n_cores)), trace=trace)
        results, extra = res.results, res

    xs = [results[c]["y"][:NPC] for c in range(n_cores)]
    out = np.concatenate(xs, axis=0)
    bsz = int(inputs["batch_size"])
    return out.reshape(bsz, -1, D).astype(np.float32), extra


def kernel(**inputs):
    out, _ = _run(inputs, trace=False)
    return out


# revision 8
# speedup vs baseline: 1.3009x; 1.2129x over previous
"""GCN+ReLU 2-layer kernel for Trainium2, 8 NeuronCores.

Strategy (dst-partitioned graph):
  - Nodes split into 8 contiguous slices; each core owns edges whose dst
    lands in its slice (host groups+sorts edges by dst once, in numpy).
  - segment_sum per 128-dst tile via one-hot selection matmuls in bf16:
    gathered src rows X_j (bf16) accumulate X_j^T @ S_j into PSUM, where
    S_j[p,q] = (oh[p,j]==q) is built on-device (iota + is_equal), one
    batched DVE op per dst tile. Aggregation runs on the *input* features
    (linearity), so the dense W matmul runs once per tile.
  - Group-packed gathers: edges are packed per (tile-group, bank) with
    padding only at segment tails; blocks spanning two tiles are matmul'd
    twice with tile-masked one-hot columns. dma_gather instructions are
    chunked at <=K_GCAP blocks (SWDGE ucode limit between 1024 and 1920
    descriptors per instruction).
  - BatchNorm folding: layer-0 output is stored RAW (pre-BN, bf16) inline
    with the loop and AllGathered; layer 1 folds the BN affine (s,t) into
    its matmuls: aggT scaled by s on PSUM copy-out, a rank-1 (W1^T t) x deg
    matmul accumulates the shift-term aggregate, and the residual uses
    diag(s) Wr1 with bias br1 + Wr1^T t. Exact algebra; lets the AllGather
    and layer-1 prep overlap layer-0's tail instead of a BN barrier.
  - Gather sources are bf16; with K_OVERFETCH=1 each descriptor fetches
    512B (two node rows, elem_step=128) since HW moves 512B rows faster
    than 256B ones; the second half of each slot is ignored.
  - Layer outputs live in SBUF transposed [feat, node]; global BN stats
    via a [128,2] AllReduce; final y = s1*x1_raw + t1 applied in place.
  - Uniform SPMD program: per-core data (edge indices, one-hot ids, degs)
    comes in as inputs; block counts/schedules are cross-core maxima and
    unions, pad slots gather row 0 with one-hot id -1.
"""
import os
import sys
sys.path.insert(0, '/opt/trn_rl_repo')

from contextlib import ExitStack

import numpy as np
import ml_dtypes

import concourse.bass as bass
import concourse.bacc as bacc_mod
import concourse.mybir as mybir
from concourse import bass_utils
from concourse.tile import TileContext

P = 128
D = 128
N_CORES = 8
N_BANKS = 4
G_TILES = 4          # dst tiles per gather group
BN_EPS = 1e-5
OVERFETCH = os.environ.get("K_OVERFETCH", "1") == "1"
GCAP = int(os.environ.get("K_GCAP", "8"))

F32 = mybir.dt.float32
BF16 = mybir.dt.bfloat16
I32 = mybir.dt.int32
I16 = mybir.dt.int16
Alu = mybir.AluOpType
Act = mybir.ActivationFunctionType


def _preprocess(src, dst, N, n_cores):
    """Group-pack edges per (group, bank) across tiles with tail padding.

    Returns per-core idx16 / oh arrays plus the shared structure:
      gnb[g][b]   blocks per (group, bank)  (cross-core max)
      sched[g]    {t: [(b, j), ...]} matmul schedule (cross-core union)
      degp_l      per-core [128, 128] in-degree table (partition=tile)
    """
    NPC = N // n_cores
    T_NODE = -(-NPC // P)
    bank_rows = -(-N // N_BANKS)
    n_groups = -(-T_NODE // G_TILES)

    order = np.argsort(dst, kind="stable")
    src_s = src[order].astype(np.int64)
    dst_s = dst[order].astype(np.int64)

    core_lo = np.searchsorted(dst_s, np.arange(n_cores) * NPC)
    core_hi = np.searchsorted(dst_s, (np.arange(n_cores) + 1) * NPC)

    # per (core, group, bank): slot arrays (src_banklocal, tile, dst_local)
    seg = {}
    for c in range(n_cores):
        s_c = src_s[core_lo[c]:core_hi[c]]
        dl_c = dst_s[core_lo[c]:core_hi[c]] - c * NPC
        t_c = dl_c // P
        g_c = t_c // G_TILES
        b_c = s_c // bank_rows
        # sort by (group, bank, tile) stable — within stays dst-sorted
        o = np.lexsort((t_c, b_c, g_c))
        s_o, dl_o, t_o, g_o, b_o = (s_c[o], dl_c[o], t_c[o], g_c[o], b_c[o])
        gb = g_o * N_BANKS + b_o
        lo = np.searchsorted(gb, np.arange(n_groups * N_BANKS))
        hi = np.searchsorted(gb, np.arange(n_groups * N_BANKS) + 1)
        for g in range(n_groups):
            for b in range(N_BANKS):
                i0, i1 = lo[g * N_BANKS + b], hi[g * N_BANKS + b]
                seg[(c, g, b)] = (s_o[i0:i1] - b * bank_rows,
                                  t_o[i0:i1], dl_o[i0:i1] - t_o[i0:i1] * P)

    gnb = [[0] * N_BANKS for _ in range(n_groups)]
    for g in range(n_groups):
        for b in range(N_BANKS):
            m = max(len(seg[(c, g, b)][0]) for c in range(n_cores))
            gnb[g][b] = -(-m // P) if m else 0

    # union schedule: per (g, b, block j) the set of tiles present
    sched = []
    for g in range(n_groups):
        tmap = {}
        for b in range(N_BANKS):
            nb = gnb[g][b]
            if nb == 0:
                continue
            present = np.zeros((nb, T_NODE), bool)
            for c in range(n_cores):
                _, t_arr, _ = seg[(c, g, b)]
                if len(t_arr) == 0:
                    continue
                blk = np.arange(len(t_arr)) // P
                present[blk, t_arr] = True
            for j in range(nb):
                for t in np.nonzero(present[j])[0]:
                    tmap.setdefault(int(t), []).append((b, j))
        sched.append({t: tmap[t] for t in sorted(tmap)})

    totblk = sum(sum(r) for r in gnb)
    totcols = totblk * 8
    totoh = sum(len(v) for s in sched for v in s.values())

    idx16_l, oh_l, degp_l = [], [], []
    for c in range(n_cores):
        idx16 = np.zeros((P, totcols), np.int16)
        oh = np.full((P, totoh), -1, np.int16)
        icol = 0
        ocol = 0
        for g in range(n_groups):
            # idx cols in (b, slot) order
            tiles_arr = {}
            for b in range(N_BANKS):
                nb = gnb[g][b]
                if nb == 0:
                    continue
                ni = nb * P
                s_arr, t_arr, d_arr = seg[(c, g, b)]
                arr = np.zeros(ni, np.int64)
                arr[:len(s_arr)] = s_arr
                tile16 = arr.reshape(ni // 16, 16).T.astype(np.int16)
                idx16[:, icol:icol + nb * 8] = np.tile(tile16, (8, 1))
                icol += nb * 8
                tpad = np.full(ni, -1, np.int64)
                tpad[:len(t_arr)] = t_arr
                dpad = np.full(ni, -1, np.int64)
                dpad[:len(d_arr)] = d_arr
                tiles_arr[b] = (tpad, dpad)
            # oh cols in schedule order (t-major)
            for t, blocks in sched[g].items():
                for (b, j) in blocks:
                    tpad, dpad = tiles_arr[b]
                    sl = slice(j * P, (j + 1) * P)
                    col = np.where(tpad[sl] == t, dpad[sl], -1)
                    oh[:, ocol] = col.astype(np.int16)
                    ocol += 1
        assert icol == totcols and ocol == totoh
        idx16_l.append(idx16)
        oh_l.append(oh)
        # in-degree of this core's dst slice, [tile partition, dst-in-tile]
        deg = np.bincount(dst_s[core_lo[c]:core_hi[c]] - c * NPC,
                          minlength=T_NODE * P).astype(np.float32)
        degp_l.append(deg.reshape(T_NODE, P))

    meta = dict(NPC=NPC, T_NODE=T_NODE, bank_rows=bank_rows, totblk=totblk,
                totcols=totcols, totoh=totoh, n_groups=n_groups)
    return idx16_l, oh_l, degp_l, gnb, sched, meta


def _build(N, gnb, sched, n_cores):
    NPC = N // n_cores
    T_NODE = -(-NPC // P)
    NPC_PAD = T_NODE * P
    bank_rows = -(-N // N_BANKS)
    n_groups = len(gnb)
    groups = [list(range(n_cores))]
    n_last = NPC - (T_NODE - 1) * P
    totblk = sum(sum(r) for r in gnb)
    totcols = totblk * 8
    totoh = sum(len(v) for s in sched for v in s.values())
    GBLK_MAX = max(sum(g) for g in gnb)
    SMAX = max(len(v) for s in sched for v in s.values())
    SLOT = 2 * D if OVERFETCH else D

    def grp_tiles(g):
        return list(range(g * G_TILES, min((g + 1) * G_TILES, T_NODE)))

    nc = bacc_mod.Bacc(num_devices=n_cores, num_swdge_queues=4)

    NPAD = 1 if OVERFETCH else 0
    hgb = nc.dram_tensor("hgb", [N + NPAD, D], BF16, kind="ExternalInput")
    hst = nc.dram_tensor("hst", [P, NPC_PAD], F32, kind="ExternalInput")
    i16d = nc.dram_tensor("i16", [P, totcols], I16, kind="ExternalInput")
    ohd = nc.dram_tensor("oh", [P, totoh], I16, kind="ExternalInput")
    degd = nc.dram_tensor("degr", [1, NPC_PAD], BF16, kind="ExternalInput")
    wb0d = nc.dram_tensor("wb0", [D, D], BF16, kind="ExternalInput")
    wr0d = nc.dram_tensor("wr0", [D, D], F32, kind="ExternalInput")
    wb1d = nc.dram_tensor("wb1", [D, D], BF16, kind="ExternalInput")
    wr1d = nc.dram_tensor("wr1", [D, D], F32, kind="ExternalInput")
    bsd = nc.dram_tensor("bs", [D, 8], F32, kind="ExternalInput")
    idnd = nc.dram_tensor("idn", [P, P], F32, kind="ExternalInput")
    yd = nc.dram_tensor("y", [NPC, D], F32, kind="ExternalOutput")

    xb = nc.dram_tensor("xb", [NPC, D], BF16)
    xg = nc.dram_tensor("xg", [n_cores * NPC + NPAD, D], BF16,
                        addr_space="Shared")
    sti = [nc.dram_tensor(f"sti{i}", [P, 2], F32) for i in range(2)]
    sto = [nc.dram_tensor(f"sto{i}", [P, 2], F32, addr_space="Shared")
           for i in range(2)]

    with TileContext(nc) as tc, ExitStack() as ctx:
        const = ctx.enter_context(tc.tile_pool(name="const", bufs=1))
        big = ctx.enter_context(tc.tile_pool(name="big", bufs=1))
        gpool = ctx.enter_context(tc.tile_pool(name="gp", bufs=2))
        hpool = ctx.enter_context(tc.tile_pool(name="hp", bufs=2))
        spool = ctx.enter_context(tc.tile_pool(name="sp", bufs=2))
        small = ctx.enter_context(tc.tile_pool(name="sm", bufs=4))
        pagg = ctx.enter_context(tc.tile_pool(name="pagg", bufs=2, space="PSUM"))
        pmm = ctx.enter_context(tc.tile_pool(name="pmm", bufs=2, space="PSUM"))
        pres = ctx.enter_context(tc.tile_pool(name="pres", bufs=2, space="PSUM"))

        def ct(shape, dtype, srcap=None, name=None):
            t = const.tile(shape, dtype, tag=name)
            if srcap is not None:
                nc.sync.dma_start(out=t[:], in_=srcap)
            return t

        wb0_t = ct([D, D], BF16, wb0d[:, :], "wb0")
        wr0_t = ct([D, D], F32, wr0d[:, :], "wr0")
        wb1_t = ct([D, D], BF16, wb1d[:, :], "wb1")
        wr1_t = ct([D, D], F32, wr1d[:, :], "wr1")
        bias_t = ct([D, 8], F32, bsd[:, :], "bs")
        ident_t = ct([P, P], F32, idnd[:, :], "idn")
        oh_t = ct([P, totoh], I16, ohd[:, :], "oh")
        i16_t = ct([P, totcols], I16, i16d[:, :], "i16")
        degr_t = ct([1, NPC_PAD], BF16, degd[:, :], "degr")
        iota_t = ct([P, P], I16, None, "iota")
        nc.gpsimd.iota(iota_t[:], pattern=[[1, P]], base=0, channel_multiplier=0)
        eps_t = ct([P, 1], F32, None, "eps")
        nc.vector.memset(eps_t[:], BN_EPS)

        xT = big.tile([P, NPC_PAD], F32, tag="xT")

        scol = [ct([P, T_NODE], F32, None, f"scol{i}") for i in range(2)]
        qcol = [ct([P, T_NODE], F32, None, f"qcol{i}") for i in range(2)]

        qctr = [0]

        def gather_ap(gsrc, lo, hi):
            """Source AP for a bank: [rows, SLOT] with row stride D
            (overlapping rows when OVERFETCH)."""
            if not OVERFETCH:
                return gsrc[lo:hi, :]
            ap = gsrc[lo:hi, :]
            return bass.AP(tensor=ap.tensor, offset=ap.offset,
                           ap=[[D, hi - lo], [1, SLOT]])

        def stats_and_affine(li, gcol, becol, scale_nm, shift_nm):
            st_sb = small.tile([P, 2], F32, tag="stats")
            nc.vector.reduce_sum(out=st_sb[:, 0:1], in_=scol[li][:],
                                 axis=mybir.AxisListType.X)
            nc.vector.reduce_sum(out=st_sb[:, 1:2], in_=qcol[li][:],
                                 axis=mybir.AxisListType.X)
            nc.sync.dma_start(out=sti[li][:, :], in_=st_sb[:])
            nc.gpsimd.collective_compute(
                "AllReduce", Alu.add, replica_groups=groups,
                ins=[sti[li].ap().opt()], outs=[sto[li].ap().opt()])
            stg = small.tile([P, 2], F32, tag="stg")
            nc.sync.dma_start(out=stg[:], in_=sto[li][:, :])
            mean = small.tile([P, 1], F32, tag="mean")
            nc.vector.tensor_scalar_mul(mean[:], stg[:, 0:1], 1.0 / N)
            ex2 = small.tile([P, 1], F32, tag="ex2")
            nc.vector.tensor_scalar_mul(ex2[:], stg[:, 1:2], 1.0 / N)
            var = small.tile([P, 1], F32, tag="var")
            nc.vector.tensor_tensor(out=var[:], in0=mean[:], in1=mean[:],
                                    op=Alu.mult)
            nc.vector.tensor_tensor(out=var[:], in0=ex2[:], in1=var[:],
                                    op=Alu.subtract)
            sd = small.tile([P, 1], F32, tag="sd")
            nc.scalar.activation(sd[:], var[:], Act.Sqrt, bias=eps_t[:, 0:1])
            rstd = small.tile([P, 1], F32, tag="rstd")
            nc.vector.reciprocal(rstd[:], sd[:])
            scale_t = ct([P, 1], F32, None, scale_nm)
            nc.vector.tensor_tensor(out=scale_t[:],
                                    in0=bias_t[:, gcol:gcol + 1],
                                    in1=rstd[:], op=Alu.mult)
            shift_t = ct([P, 1], F32, None, shift_nm)
            nc.vector.tensor_tensor(out=shift_t[:], in0=mean[:],
                                    in1=scale_t[:], op=Alu.mult)
            nc.vector.tensor_tensor(out=shift_t[:],
                                    in0=bias_t[:, becol:becol + 1],
                                    in1=shift_t[:], op=Alu.subtract)
            return scale_t, shift_t

        def store_tile(t, dt_out, out_dram, tag):
            pt = pres.tile([P, P], F32, tag="pq")
            nc.tensor.transpose(pt[:], xT[:, t * P:(t + 1) * P], ident_t[:])
            stg2 = small.tile([P, P], dt_out, tag=tag)
            nc.scalar.activation(stg2[:], pt[:], Act.Copy)
            nrow = P if t < T_NODE - 1 else n_last
            nc.sync.dma_start(out=out_dram[t * P:t * P + nrow, :],
                              in_=stg2[:nrow, :])

        def layer(li, gsrc, wb_t, bcol, brcol, out_dram,
                  res_w=None, res_bias=None, agg_scale=None, u_row=None):
            icol = 0
            ocol = 0
            for g in range(n_groups):
                tiles = grp_tiles(g)
                gt = gpool.tile([P, GBLK_MAX, SLOT], BF16, tag="g")
                if li == 0:
                    hsg = hpool.tile([P, G_TILES * P], F32, tag="hs")
                    nc.sync.dma_start(
                        out=hsg[:, :len(tiles) * P],
                        in_=hst[:, tiles[0] * P:tiles[0] * P + len(tiles) * P])
                boff = 0
                bank_pos = [0] * N_BANKS
                for b in range(N_BANKS):
                    bank_pos[b] = boff
                    lo = b * bank_rows
                    hi = min(N, lo + bank_rows)
                    src_ap = gather_ap(gsrc, lo, hi)
                    left = gnb[g][b]
                    while left > 0:
                        nb = min(left, GCAP)
                        left -= nb
                        ni = nb * P
                        nc.gpsimd.dma_gather(
                            out_ap=gt[:, boff:boff + nb, :],
                            in_ap=src_ap,
                            idxs_ap=i16_t[:, icol:icol + nb * 8],
                            num_idxs=ni,
                            num_idxs_reg=ni,
                            elem_size=SLOT,
                            elem_step=D if OVERFETCH else None,
                            queue_num=qctr[0] % 4,
                        )
                        qctr[0] += 1
                        icol += nb * 8
                        boff += nb
                for t, blocks in sched[g].items():
                    tnb = len(blocks)
                    S2 = spool.tile([P, SMAX, P], BF16, tag="S")
                    nc.vector.tensor_tensor(
                        out=S2[:, :tnb, :],
                        in0=oh_t[:, ocol:ocol + tnb].unsqueeze(2)
                            .to_broadcast([P, tnb, P]),
                        in1=iota_t[:, :].unsqueeze(1)
                            .to_broadcast([P, tnb, P]),
                        op=Alu.is_equal,
                    )
                    pa = pagg.tile([P, P], F32, tag="pa")
                    for k, (b, j) in enumerate(blocks):
                        pos = bank_pos[b] + j
                        nc.tensor.matmul(pa[:], lhsT=gt[:, pos, 0:D],
                                         rhs=S2[:, k, :],
                                         start=(k == 0), stop=(k == tnb - 1))
                    ocol += tnb
                    aggT = small.tile([P, P], BF16, tag="aggT")
                    if agg_scale is None:
                        nc.scalar.activation(aggT[:], pa[:], Act.Copy)
                    else:
                        nc.scalar.activation(aggT[:], pa[:], Act.Copy,
                                             scale=agg_scale[:, 0:1])
                    pm = pmm.tile([P, P], F32, tag="pm")
                    if u_row is None:
                        nc.tensor.matmul(pm[:], lhsT=wb_t[:], rhs=aggT[:],
                                         start=True, stop=True)
                    else:
                        nc.tensor.matmul(pm[:], lhsT=wb_t[:], rhs=aggT[:],
                                         start=True, stop=False)
                        nc.tensor.matmul(pm[:], lhsT=u_row,
                                         rhs=degr_t[0:1, t * P:(t + 1) * P],
                                         start=False, stop=True)
                    if li == 0:
                        hT = hsg[:, (t - tiles[0]) * P:(t - tiles[0] + 1) * P]
                    else:
                        hT = xT[:, t * P:(t + 1) * P]
                    newt = small.tile([P, P], F32, tag="newt")
                    nc.scalar.activation(newt[:], pm[:], Act.Relu,
                                         bias=bias_t[:, bcol:bcol + 1])
                    pr = pres.tile([P, P], F32, tag="pq")
                    nc.tensor.matmul(pr[:], lhsT=res_w[:], rhs=hT,
                                     start=True, stop=True)
                    rest = small.tile([P, P], F32, tag="rest")
                    if res_bias is None:
                        nc.scalar.activation(rest[:], pr[:], Act.Relu,
                                             bias=bias_t[:, brcol:brcol + 1])
                    else:
                        nc.scalar.activation(rest[:], pr[:], Act.Relu,
                                             bias=res_bias[:, 0:1])
                    ov = xT[:, t * P:(t + 1) * P]
                    if t == T_NODE - 1 and n_last < P:
                        nc.vector.scalar_tensor_tensor(
                            out=ov, in0=newt[:], scalar=0.0, in1=rest[:],
                            op0=Alu.add, op1=Alu.add)
                        nc.vector.memset(xT[:, t * P + n_last:(t + 1) * P], 0.0)
                        nc.vector.reduce_sum(out=scol[li][:, t:t + 1], in_=ov,
                                             axis=mybir.AxisListType.X)
                    else:
                        nc.vector.scalar_tensor_tensor(
                            out=ov, in0=newt[:], scalar=0.0, in1=rest[:],
                            op0=Alu.add, op1=Alu.add,
                            accum_out=scol[li][:, t:t + 1])
                    sq = small.tile([P, P], F32, tag="sq")
                    nc.scalar.activation(sq[:], ov, Act.Square,
                                         accum_out=qcol[li][:, t:t + 1])
                    if li == 0:
                        # store raw bf16 immediately (pre-BN; BN folded
                        # into layer 1)
                        store_tile(t, BF16, out_dram, "stage0")
            assert ocol == totoh and icol == totcols

        # ---- layer 0 (raw out, stores inline) ----
        layer(0, hgb, wb0_t, 0, 1, xb, res_w=wr0_t)
        nc.gpsimd.collective_compute(
            "AllGather", Alu.bypass, replica_groups=groups,
            ins=[xb.ap().opt()], outs=[xg[0:n_cores * NPC, :].opt()])
        s0, t0 = stats_and_affine(0, 2, 3, "s0", "t0")

        # ---- layer-1 affine prep (overlaps AllGather) ----
        t0b = ct([P, 1], BF16, None, "t0b")
        nc.vector.tensor_copy(t0b[:], t0[:])
        pu = pres.tile([P, P], F32, tag="pq")
        nc.tensor.matmul(pu[0:1, :], lhsT=t0b[:], rhs=wb1_t[:],
                         start=True, stop=True)
        u_sb = ct([1, P], BF16, None, "u_sb")
        nc.scalar.activation(u_sb[:], pu[0:1, :], Act.Copy)
        wr1s = ct([D, D], F32, None, "wr1s")
        nc.vector.tensor_scalar(out=wr1s[:], in0=wr1_t[:],
                                scalar1=s0[:, 0:1], scalar2=0.0,
                                op0=Alu.mult, op1=Alu.add)
        pbr = pres.tile([P, P], F32, tag="pq")
        nc.tensor.matmul(pbr[:, 0:1], lhsT=wr1_t[:], rhs=t0[:],
                         start=True, stop=True)
        br1p = ct([P, 1], F32, None, "br1p")
        nc.vector.tensor_tensor(out=br1p[:], in0=bias_t[:, 5:6],
                                in1=pbr[:, 0:1], op=Alu.add)

        # ---- layer 1 (BN-folded consumption of raw xg) ----
        layer(1, xg, wb1_t, 4, 5, yd,
              res_w=wr1s, res_bias=br1p, agg_scale=s0, u_row=u_sb[0:1, :])
        s1, t1 = stats_and_affine(1, 6, 7, "s1", "t1")
        nc.vector.tensor_scalar(
            out=xT[:, :], in0=xT[:, :],
            scalar1=s1[:, 0:1], scalar2=t1[:, 0:1],
            op0=Alu.mult, op1=Alu.add)
        for t in range(T_NODE):
            store_tile(t, F32, yd, "stage1")
    nc.compile()
    return nc


def _run(inputs, n_cores=N_CORES, trace=False, runner=None):
    h = np.asarray(inputs["h"], np.float32)
    src = np.asarray(inputs["src"])
    dst = np.asarray(inputs["dst"])
    N = h.shape[0]
    NPC = N // n_cores
    idx16_l, oh_l, degp_l, gnb, sched, meta = _preprocess(src, dst, N, n_cores)
    T_NODE = meta["T_NODE"]
    NPC_PAD = T_NODE * P

    nc = _build(N, gnb, sched, n_cores)

    bs = np.stack([
        np.asarray(inputs["b0"], np.float32),
        np.asarray(inputs["br0"], np.float32),
        np.asarray(inputs["g0"], np.float32),
        np.asarray(inputs["be0"], np.float32),
        np.asarray(inputs["b1"], np.float32),
        np.asarray(inputs["br1"], np.float32),
        np.asarray(inputs["g1"], np.float32),
        np.asarray(inputs["be1"], np.float32),
    ], axis=1)
    idn = np.eye(P, dtype=np.float32)
    if OVERFETCH:
        hgb = np.zeros((N + 1, D), ml_dtypes.bfloat16)
        hgb[:N] = h.astype(ml_dtypes.bfloat16)
    else:
        hgb = h.astype(ml_dtypes.bfloat16)

    in_maps = []
    for c in range(n_cores):
        hst_c = np.zeros((P, NPC_PAD), np.float32)
        hst_c[:, :NPC] = h[c * NPC:(c + 1) * NPC].T
        degr_c = np.zeros((1, NPC_PAD), np.float32)
        degr_c[0, :] = degp_l[c].reshape(-1)
        in_maps.append({
            "hgb": hgb,
            "hst": hst_c,
            "i16": idx16_l[c],
            "oh": oh_l[c],
            "degr": degr_c.astype(ml_dtypes.bfloat16),
            "wb0": np.asarray(inputs["W0"], np.float32).astype(ml_dtypes.bfloat16),
            "wr0": np.asarray(inputs["Wr0"], np.float32),
            "wb1": np.asarray(inputs["W1"], np.float32).astype(ml_dtypes.bfloat16),
            "wr1": np.asarray(inputs["Wr1"], np.float32),
            "bs": bs,
            "idn": idn,
        })

    if runner is not None:
        results, extra = runner(nc, in_maps)
    else:
        res = bass_utils.run_bass_kernel_spmd(
            nc, in_maps, core_ids=list(range(n_cores)), trace=trace)
        results, extra = res.results, res

    xs = [results[c]["y"][:NPC] for c in range(n_cores)]
    out = np.concatenate(xs, axis=0)
    bsz = int(inputs["batch_size"])
    return out.reshape(bsz, -1, D).astype(np.float32), extra


def kernel(**inputs):
    out, _ = _run(inputs, trace=False)
    return out
